# revision 1
# baseline (speedup 1.0000x reference)
"""Multi-head self-attention with RoPE — 8-core SPMD Bass kernel for TRN2.

Problem: nn_MultiHeadSelfAttention (b=2, s=2048, d=1024, h=16, hd=64),
y = softmax(mask(RoPE(xWq^T) RoPE(xWk^T)^T / 8)) (xWv^T) Wo^T.

Sharding (tensor/head parallel): heads 2i, 2i+1 -> core i. Each core
computes Q/K/V projections for its 2 heads over the full sequence (both
batches), applies RoPE, computes attention, AllGathers the per-head
attention outputs (transposed [d, seq] layout, fp16), and computes a
128-column slice of the output projection. The host assembles the 8
column slices into the full output.

Kernel layout/engineering notes:
 - All matmul-adjacent tensors are dtype float32r (fp32 bytes, TF32-like
   matmul at full PE rate for moving dim >= 256).
 - Activations are kept transposed ([d, seq]) end-to-end so every matmul
   has its contraction on the partition axis with N=512 moving columns;
   no transposes are needed except V (32 cheap PE-transposes).
 - hd components of Q/K are host-deinterleaved (evens then odds) so the
   RoPE partner lives at a fixed partition offset; the partner shuffle is
   one PE matmul against a constant +-1 permutation matrix, and RoPE is
   3 DVE ops against host-precomputed cos/sin tables. The 1/sqrt(hd)
   scale is folded into Wq on the host.
 - Scores are computed transposed (ST[k, q]) so the key-padding mask is
   a per-partition ACT bias on the exp activation (exp(score - 30) for
   masked keys ~ 0), and the softmax denominators ride the PV matmul as
   a 65th all-ones column of V (row 64 of the PV output = row sums).
   Normalization is deferred to the [64, 512] attention outputs.
 - The two heads' QK^T matmuls have K=64 and sit at partition bases 0/64,
   so the PE row-groups run them concurrently.
 - Batches are software-pipelined in a wavefront: attention chunks are
   emitted as soon as their Q/K column blocks exist, so the ACT engine
   (exp is the 109us/core floor) starts ~20us in and runs ~95% duty.
   AllGather(b0) overlaps batch-1 compute; only AG(b1) + its out-proj
   remain exposed in the tail.
"""

from contextlib import ExitStack

import numpy as np

import concourse.bacc as bacc_mod
import concourse.tile as tile
from concourse import mybir
from concourse.bass_utils import run_bass_kernel_spmd

F32 = mybir.dt.float32
F32R = mybir.dt.float32r
F16 = mybir.dt.float16
EXP = mybir.ActivationFunctionType.Exp

B = 2
S = 2048
D = 1024
H = 16
HD = 64
NCORES = 8
HPC = H // NCORES          # 2 heads per core
S2 = B * S                 # 4096
NCB = S2 // 512            # 8 column blocks of 512
NCBB = S // 512            # 4 column blocks per batch
NKT = S // 128             # 16 key tiles per batch
NQB = S // 512             # 4 query blocks per batch
DKT = D // 128             # 8 contraction tiles for d=1024
MASK_NEG = -30.0


def build_kernel(repeats: int = 1):
    nc = bacc_mod.Bacc("TRN2", target_bir_lowering=False, debug=False,
                       num_devices=NCORES)

    xT = nc.declare_dram_parameter("xT", [D, S2], F32R, isOutput=False)
    wq = nc.declare_dram_parameter("wq", [D, 128], F32R, isOutput=False)
    wk = nc.declare_dram_parameter("wk", [D, 128], F32R, isOutput=False)
    wv = nc.declare_dram_parameter("wv", [D, 128], F32R, isOutput=False)
    wo = nc.declare_dram_parameter("wo", [D, 128], F16, isOutput=False)
    cosT = nc.declare_dram_parameter("cosT", [128, S2], F32R, isOutput=False)
    sinT = nc.declare_dram_parameter("sinT", [128, S2], F32R, isOutput=False)
    perm = nc.declare_dram_parameter("perm", [128, 128], F32R, isOutput=False)
    ident = nc.declare_dram_parameter("ident", [128, 128], F32R, isOutput=False)
    maskb = nc.declare_dram_parameter("maskb", [128, B * NKT], F32,
                                      isOutput=False)
    onesc = nc.declare_dram_parameter("onesc", [128, B * HPC * NKT], F32R,
                                      isOutput=False)
    out = nc.declare_dram_parameter("out", [128, S2], F32, isOutput=True)

    xT_t = xT.rearrange("(kt p) c -> p kt c", p=128)
    w_t = {n: w.rearrange("(kt p) m -> p kt m", p=128)
           for n, w in (("wq", wq), ("wk", wk), ("wv", wv), ("wo", wo))}

    with tile.TileContext(nc) as tc:
        for _ in range(repeats):
            _emit_body(nc, tc, xT_t, w_t, cosT, sinT, perm, ident, maskb,
                       onesc, out)
    nc.compile()
    return nc


def _emit_body(nc, tc, xT_t, w_t, cosT, sinT, perm, ident, maskb, onesc, out):
    with ExitStack() as body:
        consts = body.enter_context(tc.tile_pool(name="consts", bufs=1))
        w_sb = {}
        for n in ("wq", "wk", "wv"):
            w_sb[n] = consts.tile([128, DKT, 128], F32R, name=f"{n}_sb")
            nc.sync.dma_start(out=w_sb[n], in_=w_t[n])
        w_sb["wo"] = consts.tile([128, DKT, 128], F16, name="wo_sb")
        perm_sb = consts.tile([128, 128], F32R)
        nc.sync.dma_start(out=perm_sb, in_=perm[:, :])
        ident_sb = consts.tile([128, 128], F32R)
        nc.sync.dma_start(out=ident_sb, in_=ident[:, :])
        maskb_sb = consts.tile([128, B * NKT], F32)
        nc.sync.dma_start(out=maskb_sb, in_=maskb[:, :])

        # persistent activations (transposed layouts)
        acts = body.enter_context(tc.tile_pool(name="acts", bufs=1))
        qrot = acts.tile([128, NCB, 512], F32R)
        krot = acts.tile([128, NCB, 512], F32R)
        v_all = acts.tile([128, B * HPC, NKT, 65], F32R)
        nc.sync.dma_start(out=v_all[:, :, :, 64],
                          in_=onesc.rearrange("p (a k) -> p a k", a=B * HPC))

        # pools with whole-kernel lifetime; PSUM banks are statically
        # partitioned (proj 2 + perm/transpose 1 + scores 3 + pv 1 + out 1)
        st_ps = body.enter_context(
            tc.tile_pool(name="st_ps", bufs=3, space="PSUM"))
        o_ps = body.enter_context(
            tc.tile_pool(name="o_ps", bufs=1, space="PSUM"))
        fo_ps = body.enter_context(
            tc.tile_pool(name="fo_ps", bufs=1, space="PSUM"))
        ppool = body.enter_context(tc.tile_pool(name="ppool", bufs=6))
        npool = body.enter_context(tc.tile_pool(name="npool", bufs=2))
        uaccp = body.enter_context(tc.tile_pool(name="uaccp", bufs=8))
        u16p = body.enter_context(tc.tile_pool(name="u16p", bufs=8))
        upool = body.enter_context(tc.tile_pool(name="upool", bufs=8))
        opool = body.enter_context(tc.tile_pool(name="opool", bufs=2))
        dram = body.enter_context(
            tc.tile_pool(name="dram", bufs=1, space="DRAM"))

        cc_out = {}
        u_acc = {}
        u_16 = {}

        def emit_B_cb(b, c):
            """Q/K/V projections + RoPE for column block c of batch b."""
            cb = b * NCBB + c
            xsb = xpool.tile([128, DKT, 512], F32R, tag="xsb", name="xsb")
            nc.sync.dma_start(out=xsb[:, 0:4, :],
                              in_=xT_t[:, 0:4, cb * 512:(cb + 1) * 512])
            nc.sync.dma_start(out=xsb[:, 4:8, :],
                              in_=xT_t[:, 4:8, cb * 512:(cb + 1) * 512])
            cos_cb = cpool.tile([128, 512], F32R, tag="cos", name="cos_cb")
            nc.sync.dma_start(out=cos_cb, in_=cosT[:, cb * 512:(cb + 1) * 512])
            sin_cb = cpool.tile([128, 512], F32R, tag="sin", name="sin_cb")
            nc.sync.dma_start(out=sin_cb, in_=sinT[:, cb * 512:(cb + 1) * 512])
            for name, dst, rope in (("wv", vt[b], False),
                                    ("wq", qrot, True),
                                    ("wk", krot, True)):
                pr = proj_ps.tile([128, 512], F32, tag="proj", name="pr")
                for kt in range(DKT):
                    nc.tensor.matmul(pr, w_sb[name][:, kt, :], xsb[:, kt, :],
                                     start=(kt == 0), stop=(kt == DKT - 1))
                if not rope:
                    nc.vector.tensor_copy(dst[:, c, :], pr)
                    continue
                raw = tmp.tile([128, 512], F32R, tag="raw", name="raw")
                nc.vector.tensor_copy(raw, pr)
                pp = proj_ps.tile([128, 512], F32, tag="pp", name="pp", bufs=1)
                nc.tensor.matmul(pp, perm_sb, raw, start=True, stop=True)
                tcos = tmp.tile([128, 512], F32R, tag="tcos", name="tcos")
                nc.vector.tensor_mul(tcos, raw, cos_cb)
                tsin = tmp.tile([128, 512], F32R, tag="tsin", name="tsin")
                nc.vector.tensor_mul(tsin, pp, sin_cb)
                nc.vector.tensor_add(dst[:, cb, :], tcos, tsin)
            # transpose this block's V into [k, hd] layout (4 key tiles)
            for kt in range(4 * c, 4 * c + 4):
                off = (kt % 4) * 128
                tp = proj_ps.tile([128, 128], F32R, tag="pp", name="tp",
                                  bufs=1)
                nc.tensor.transpose(tp, vt[b][:, c, off:off + 128], ident_sb)
                for ln in range(HPC):
                    nc.vector.tensor_copy(
                        v_all[:, b * HPC + ln, kt, 0:64],
                        tp[:, ln * 64:(ln + 1) * 64])

        def emit_C_chunk(b, qb, j):
            """Attention chunk: query block qb vs key tiles 4j..4j+3."""
            cb_q = b * NQB + qb
            for ln in range(HPC):
                if j == 0:
                    u_acc[(b, qb, ln)] = uaccp.tile(
                        [65, 512], F32, tag="uacc", name=f"ua{qb}{ln}")
                oc = o_ps.tile([65, 512], F32, tag="oc", name=f"oc{ln}")
                for kt in range(4 * j, 4 * j + 4):
                    cb_k, off = divmod(b * S + kt * 128, 512)
                    mb = maskb_sb[:, (b * NKT + kt):(b * NKT + kt) + 1]
                    st = st_ps.tile([128, 512], F32, tag="st", name=f"st{ln}")
                    nc.tensor.matmul(
                        st,
                        krot[ln * 64:(ln + 1) * 64, cb_k, off:off + 128],
                        qrot[ln * 64:(ln + 1) * 64, cb_q, :],
                        start=True, stop=True)
                    p = ppool.tile([128, 512], F32R, tag="p", name=f"p{ln}")
                    nc.scalar.activation(p, st, EXP, bias=mb, scale=1.0)
                    nc.tensor.matmul(
                        oc, v_all[:, b * HPC + ln, kt, :], p,
                        start=(kt == 4 * j), stop=(kt == 4 * j + 3))
                ua = u_acc[(b, qb, ln)]
                if j == 0:
                    nc.vector.tensor_copy(ua, oc)
                else:
                    nc.vector.tensor_add(ua, ua, oc)

        def emit_norm(b, qb):
            for ln in range(HPC):
                ua = u_acc[(b, qb, ln)]
                rec = npool.tile([1, 512], F32, tag="rec", name=f"rec{ln}")
                nc.vector.reciprocal(rec, ua[64:65, :])
                recb = npool.tile([64, 512], F32, tag="recb", name=f"recb{ln}")
                nc.gpsimd.partition_broadcast(recb, rec)
                u16 = u16p.tile([64, 512], F16, tag="u16", name=f"u16_{qb}{ln}")
                u_16[(b, qb, ln)] = u16
                nc.vector.tensor_mul(u16, ua[0:64, :], recb)

        def emit_AG(b, bs):
            W = S * len(bs)
            cc_in = dram.tile([128, W], F16, tag="cc_in", name=f"ccin{b}")
            for bb in bs:
                for ln in range(HPC):
                    for qb in range(NQB):
                        nc.sync.dma_start(
                            out=cc_in[ln * 64:(ln + 1) * 64,
                                      (bb - bs[0]) * S + qb * 512:
                                      (bb - bs[0]) * S + (qb + 1) * 512],
                            in_=u_16[(bb, qb, ln)])
            cc_out[b] = dram.tile([D, W], F16, tag="cc_out",
                                  name=f"ccout{b}", addr_space="Shared")
            nc.gpsimd.collective_compute(
                "AllGather", mybir.AluOpType.bypass,
                replica_groups=[list(range(NCORES))],
                ins=[cc_in.opt()], outs=[cc_out[b].opt()])

        def emit_wave(b):
            for c in range(NCBB):
                emit_B_cb(b, c)
                for j in range(c + 1):
                    emit_C_chunk(b, c, j)       # (qb=c, kt-chunk j)
                for q in range(c):
                    emit_C_chunk(b, q, c)       # (qb=q, kt-chunk c)
            for qb in range(NQB):
                emit_norm(b, qb)
            emit_AG(b, [b])

        def emit_D(b, hb):
            uqs = []
            for kt in range(DKT):
                uq = upool.tile([128, 1024], F16, tag="uq", name="uq")
                nc.sync.dma_start(
                    out=uq,
                    in_=cc_out[b][kt * 128:(kt + 1) * 128,
                                  hb * 1024:(hb + 1) * 1024])
                uqs.append(uq)
            for i in range(2):
                qb = hb * 2 + i
                fo = fo_ps.tile([128, 512], F32, tag="fo", name="fo")
                for kt in range(DKT):
                    nc.tensor.matmul(fo, w_sb["wo"][:, kt, :],
                                     uqs[kt][:, i * 512:(i + 1) * 512],
                                     start=(kt == 0), stop=(kt == DKT - 1))
                osb = opool.tile([128, 512], F32, tag="osb", name="osb")
                nc.vector.tensor_copy(osb, fo)
                nc.sync.dma_start(
                    out=out[:, b * S + qb * 512: b * S + (qb + 1) * 512],
                    in_=osb)

        with ExitStack() as bphase:
            xpool = bphase.enter_context(tc.tile_pool(name="xpool", bufs=2))
            cpool = bphase.enter_context(tc.tile_pool(name="cpool", bufs=2))
            tmp = bphase.enter_context(tc.tile_pool(name="tmp", bufs=3))
            vtp = bphase.enter_context(tc.tile_pool(name="vtp", bufs=1))
            proj_ps = bphase.enter_context(
                tc.tile_pool(name="proj_ps", bufs=2, space="PSUM"))
            vt = {b: vtp.tile([128, NCBB, 512], F32R, tag="vt", name=f"vt{b}")
                  for b in range(B)}

            emit_wave(0)
            nc.sync.dma_start(out=w_sb["wo"], in_=w_t["wo"])
            emit_wave(1)

            emit_D(0, 0)
            emit_D(0, 1)
            emit_D(1, 0)
            emit_D(1, 1)


# ---------------- host-side shard prep / unshard ----------------

def prep_inputs(x, attn_mask, Wq, Wk, Wv, Wo):
    """Full inputs -> list of 8 per-core input dicts."""
    x = np.asarray(x, dtype=np.float32)
    Wq = np.asarray(Wq, dtype=np.float32)
    Wk = np.asarray(Wk, dtype=np.float32)
    Wv = np.asarray(Wv, dtype=np.float32)
    Wo = np.asarray(Wo, dtype=np.float32)
    attn_mask = np.asarray(attn_mask)

    xT = np.ascontiguousarray(x.reshape(S2, D).T)          # [1024, 4096]

    # deinterleave: even hd components then odd, within each head
    comp = np.concatenate([np.arange(0, HD, 2), np.arange(1, HD, 2)])  # [64]
    half = HD // 2
    pi = np.concatenate([np.arange(half), np.arange(half)])            # [64]
    freq = np.float32(10000.0) ** (-2.0 * pi.astype(np.float32) / HD)
    pos = np.arange(S, dtype=np.float32)
    ang = pos[None, :] * freq[:, None]                     # [64, 2048]
    cos1 = np.cos(ang).astype(np.float32)
    sin1 = np.sin(ang).astype(np.float32)
    cosT = np.ascontiguousarray(
        np.tile(np.concatenate([cos1, cos1], axis=0), (1, B)))  # [128, 4096]
    sinT = np.ascontiguousarray(
        np.tile(np.concatenate([sin1, sin1], axis=0), (1, B)))

    permM = np.zeros((128, 128), dtype=np.float32)   # perm[p_in, p_out]
    for ln in range(HPC):
        base = ln * 64
        for j in range(half):
            permM[base + half + j, base + j] = -1.0
            permM[base + j, base + half + j] = 1.0
    identM = np.eye(128, dtype=np.float32)

    maskbM = np.zeros((128, B * NKT), dtype=np.float32)
    for b in range(B):
        for kt in range(NKT):
            mslice = attn_mask[b, kt * 128:(kt + 1) * 128]
            maskbM[:, b * NKT + kt] = np.where(
                mslice, np.float32(MASK_NEG), 0.0)

    in_maps = []
    for i in range(NCORES):
        heads = [HPC * i + ln for ln in range(HPC)]
        rows_qk = np.concatenate([h * HD + comp for h in heads])      # [128]
        rows_v = np.concatenate(
            [np.arange(h * HD, (h + 1) * HD) for h in heads])
        wq_i = np.ascontiguousarray((Wq[rows_qk, :] / 8.0).T)    # [1024, 128]
        wk_i = np.ascontiguousarray(Wk[rows_qk, :].T)
        wv_i = np.ascontiguousarray(Wv[rows_v, :].T)
        wo_i = np.ascontiguousarray(
            Wo[i * 128:(i + 1) * 128, :].T.astype(np.float16))
        in_maps.append({
            "xT": xT, "wq": wq_i, "wk": wk_i, "wv": wv_i, "wo": wo_i,
            "cosT": cosT, "sinT": sinT, "perm": permM, "ident": identM,
            "maskb": maskbM,
            "onesc": np.ones((128, B * HPC * NKT), dtype=np.float32),
        })
    return in_maps


def assemble_output(results):
    """list of per-core result dicts -> full [B, S, D] output."""
    cat = np.concatenate([results[i]["out"] for i in range(NCORES)], axis=0)
    # cat[n, b*S+s] -> out[b, s, n]
    return np.ascontiguousarray(cat.reshape(D, B, S).transpose(1, 2, 0))


_NC_CACHE = {}


def kernel(x, attn_mask, Wq, Wk, Wv, Wo):
    """Full-input, full-output entry point (shards across 8 NeuronCores)."""
    if "nc" not in _NC_CACHE:
        _NC_CACHE["nc"] = build_kernel()
    nc = _NC_CACHE["nc"]
    in_maps = prep_inputs(x, attn_mask, Wq, Wk, Wv, Wo)
    res = run_bass_kernel_spmd(nc, in_maps, core_ids=list(range(NCORES)))
    return assemble_output(res.results)

